# revision 2
# baseline (speedup 1.0000x reference)

# Trainium2 Bass kernel for nn_CameraAwareLoss (self-contained), v2.
#
# Strategy (8 NeuronCores, data-parallel over groups):
#   - 16384 rows = 4096 groups x 4 samples, d=1024. Each core owns 512 groups.
#   - Per group ship only TWO fp8 1024-vectors: SA = sum of normalized rows,
#     SROW = sum of camera-sign-scrambled normalized rows. For a pair (a,b):
#     sum_c Ca_c.Cb_c ~= SROW_a.SROW_b (exact camera-matched part + zero-mean
#     noise), so the 8x8 cross-camera stat needs just 2 fused dots.
#   - ma (per-group cross-camera mean) ~= (|SROW|^2 - |SA|^2)/c1 from f32
#     PSUM self-dots, shipped as an fp8 hi/lo pair in a ghost row.
#   - AllGather 1: fc^T fp8 (0.5MB/rank); dist matmul fp8 DoubleRow with the
#     -15*same-label mask folded into PSUM; fp16 sim drain; MAX8/FIND_INDEX8.
#   - AllGather 2: [SA;SROW;ma] payload (1MB/rank), overlapped with dist.
#   - Tail: indirect gathers + 2 fused dots per tile + stats batched [128,4].
import numpy as np
import ml_dtypes

import concourse.bass as bass
import concourse.mybir as mybir
import concourse.bacc as bacc
from concourse import tile
from concourse.bass_utils import run_bass_kernel_spmd

NCORES = 8
NG = 4096          # total groups
G = NG // NCORES   # groups per core (512)
D = 1024
R = G * 4          # rows per core (2048)
RT = R // 128      # row tiles per core (16)
PT = G // 128      # group tiles per core (4)
KT = D // 128      # contraction tiles (8)
MARGIN = 0.3
MNEG = -15.0       # same-label mask value (fp8-exact, dominates cos<=1)
# single payload: rows 0..511 fc^T blocks, 512..1535 [SA;SROW] groups,
# 1536 ma hi/lo pairs, 1537..1539 pad
PAY_ROWS = 1540
EA0 = 512           # first [SA;SROW] row
MA_ROW = 1536
EA_ROWS = PAY_ROWS - EA0         # 1028
GB_RM = EA_ROWS // 2 - 512       # 2
GB_OFF = 0
MA_RM = EA_ROWS * 512 - 512      # 525824
MA_OFF = 1024 * 512              # 524288

f32 = mybir.dt.float32
bf16 = mybir.dt.bfloat16
fp16 = mybir.dt.float16
fp8 = mybir.dt.float8e4
u32 = mybir.dt.uint32

_CACHE = {}

AF = mybir.ActivationFunctionType
OP = mybir.AluOpType
USE_DR = True
DR = mybir.MatmulPerfMode.DoubleRow


def _build():
    nc = bacc.Bacc("TRN2", target_bir_lowering=False, debug=False,
                   num_devices=NCORES)

    x_sh = nc.dram_tensor("x_sh", [R, D], bf16, kind="ExternalInput")
    sgn_in = nc.dram_tensor("sgn", [R, D], fp8, kind="ExternalInput")
    e4b_in = nc.dram_tensor("e4b", [128, 4 * 128], bf16, kind="ExternalInput")
    wsa_in = nc.dram_tensor("wsa", [128, RT * 128], bf16, kind="ExternalInput")
    wsr_in = nc.dram_tensor("wsr", [128, RT * 128], bf16, kind="ExternalInput")
    red2_in = nc.dram_tensor("red2", [128, 256], f32, kind="ExternalInput")
    ident_in = nc.dram_tensor("ident", [128, 128], bf16, kind="ExternalInput")
    maskI_in = nc.dram_tensor("maskI", [128, 128], fp8, kind="ExternalInput")
    eqm_in = nc.dram_tensor("eqm", [128, PT * NG], fp8, kind="ExternalInput")
    atab_in = nc.dram_tensor("atab", [128, 16 * PT], f32, kind="ExternalInput")
    va4_in = nc.dram_tensor("va4", [128, PT], f32, kind="ExternalInput")
    gmeta = nc.dram_tensor("gmeta", [NG, 8], fp8, kind="ExternalInput")

    loss_out = nc.dram_tensor("loss_part", [1, 1], f32, kind="ExternalOutput")

    pay_all = nc.dram_tensor("pay_all", [PAY_ROWS, D], fp8, kind="Internal")
    full_fc = nc.dram_tensor("full_fc", [NCORES * EA0, D], fp8,
                             kind="Internal", addr_space="Shared")
    full_ea = nc.dram_tensor("full_ea", [NCORES * (PAY_ROWS - EA0), D], fp8,
                             kind="Internal", addr_space="Shared")

    rg = [list(range(NCORES))]

    from contextlib import ExitStack
    with tile.TileContext(nc) as tc:
        with ExitStack() as st0:
            ep0 = st0.enter_context
            ct = ep0(tc.tile_pool(name="consts", bufs=1))
            pfcT = ep0(tc.tile_pool(name="pfcT", bufs=KT // 2))
            pga = ep0(tc.tile_pool(name="pga", bufs=PT))
            pma = ep0(tc.tile_pool(name="pma", bufs=1))
            psim = ep0(tc.tile_pool(name="psim", bufs=2))
            prhs = ep0(tc.tile_pool(name="prhs", bufs=NCORES))
            peqm = ep0(tc.tile_pool(name="peqm", bufs=1))
            pgb = ep0(tc.tile_pool(name="pgb", bufs=PT))
            psmall = ep0(tc.tile_pool(name="psmall", bufs=4))
            pbat = ep0(tc.tile_pool(name="pbat", bufs=1))
            pscr = ep0(tc.tile_pool(name="pscr", bufs=2))
            pdram = ep0(tc.tile_pool(name="pdram", bufs=1, space="DRAM"))

            # ---- constants ----
            e4b = ct.tile([128, 4 * 128], bf16, tag="e4b")
            nc.sync.dma_start(e4b[:], e4b_in[:])
            wsa = ct.tile([128, RT * 128], bf16, tag="wsa")
            wsr = ct.tile([128, RT * 128], bf16, tag="wsr")
            nc.sync.dma_start(wsa[:], wsa_in[:])
            nc.sync.dma_start(wsr[:], wsr_in[:])
            red2 = ct.tile([128, 256], f32, tag="red2")
            nc.sync.dma_start(red2[:], red2_in[:])
            ident_sb = ct.tile([128, 128], bf16, tag="ident")
            nc.sync.dma_start(ident_sb[:], ident_in[:])
            maskI = ct.tile([128, 128], fp8, tag="maskI")
            nc.sync.dma_start(maskI[:], maskI_in[:])
            atab = ct.tile([128, 16 * PT], f32, tag="atab")
            nc.sync.dma_start(atab[:], atab_in[:])
            va4 = ct.tile([128, PT], f32, tag="va4")
            nc.sync.dma_start(va4[:], va4_in[:])
            eqm_t = peqm.tile([128, PT * NG], fp8, tag="eqm")
            nc.sync.dma_start(eqm_t[:], eqm_in[:])
            eqm = [eqm_t[:, NG * p:NG * (p + 1)] for p in range(PT)]
            # paired (DoubleRow) fcT tiles: j covers kk = 2j, 2j+1
            fcT = [pfcT.tile([128, 2 * G], fp8, tag="fcT", name=f"fcT{j}")
                   for j in range(KT // 2)]
            gAs = []
            ma4 = pma.tile([128, PT], f32, tag="ma4")

            dscr = pdram.tile([4 * 128, 1], f32, tag="dscr")

            # ---- phase A/B fused ----
            with ExitStack() as stA:
                epA = stA.enter_context
                px = epA(tc.tile_pool(name="px", bufs=16))
                psg = epA(tc.tile_pool(name="psg", bufs=6))
                pfa = epA(tc.tile_pool(name="pfa", bufs=6))
                pfa2 = epA(tc.tile_pool(name="pfa2", bufs=6))
                psq = epA(tc.tile_pool(name="psq", bufs=4))
                pea = epA(tc.tile_pool(name="pea", bufs=6))
                pnrm = epA(tc.tile_pool(name="pnrm", bufs=12))
                pfc = epA(tc.tile_pool(name="pfc", bufs=2))
                ppb = epA(tc.tile_pool(name="ppb", bufs=2, space="PSUM"))
                ppt = epA(tc.tile_pool(name="ppt", bufs=2, space="PSUM"))
                ppm = epA(tc.tile_pool(name="ppm", bufs=1, space="PSUM"))

                # pass 1: load x, centers, fc, ship fc (gates AG1)
                xts = []
                for rt in range(RT):
                    x_t = px.tile([128, D], bf16, tag="x", name=f"x{rt}")
                    nc.sync.dma_start(x_t[:], x_sh[128 * rt:128 * (rt + 1), :])
                    xts.append(x_t)
                for p in range(PT):
                    pc = ppb.tile([128, D], f32, tag="pbig", name=f"pc{p}")
                    for q in range(4):
                        x_t = xts[4 * p + q]
                        for h in range(2):
                            nc.tensor.matmul(pc[:, 512 * h:512 * (h + 1)],
                                             lhsT=e4b[:, 128 * q:128 * (q + 1)],
                                             rhs=x_t[:, 512 * h:512 * (h + 1)],
                                             start=(q == 0), stop=(q == 3))
                    cn2 = pnrm.tile([128, 1], f32, tag="cn2")
                    csq = psq.tile([128, D], bf16, tag="sqscr")
                    nc.scalar.activation(csq[:], pc[:], AF.Square,
                                         accum_out=cn2[:])
                    cnm = pnrm.tile([128, 1], f32, tag="cnm")
                    nc.scalar.activation(cnm[:], cn2[:], AF.Sqrt)
                    crn = pnrm.tile([128, 1], f32, tag="crn")
                    nc.vector.reciprocal(crn[:], cnm[:])
                    fc_t = pfc.tile([128, D], bf16, tag="fc")
                    nc.scalar.activation(fc_t[:], pc[:], AF.Copy, scale=crn[:])
                    for kk in range(KT):
                        tp_ps = ppt.tile([128, 128], bf16, tag="tp")
                        nc.tensor.transpose(tp_ps[:],
                                            fc_t[:, 128 * kk:128 * (kk + 1)],
                                            ident_sb[:])
                        nc.vector.tensor_copy(
                            fcT[kk // 2][:, G * (kk % 2) + 128 * p:
                                         G * (kk % 2) + 128 * (p + 1)],
                            tp_ps[:])
                for j in range(KT // 2):
                    nc.sync.dma_start(
                        pay_all[0:EA0, :].rearrange(
                            "(b j) c -> b j c", j=KT // 2)[:, j, :],
                        fcT[j][:])

                # pass 2: normalize rows, SA/SROW blocks, ma (under AG1)
                fas, fa2s = [], []
                for rt in range(RT):
                    x_t = xts[rt]
                    sg_t = psg.tile([128, D], fp8, tag="sg", name=f"sg{rt}")
                    nc.sync.dma_start(sg_t[:],
                                      sgn_in[128 * rt:128 * (rt + 1), :])
                    ssq = pnrm.tile([128, 1], f32, tag="ssq")
                    sq_scr = psq.tile([128, D], bf16, tag="sqscr")
                    if rt % 2 == 0:
                        nc.vector.scalar_tensor_tensor(
                            sq_scr[:], x_t[:], 1.0, x_t[:],
                            OP.mult, OP.mult, accum_out=ssq[:])
                    else:
                        nc.scalar.activation(sq_scr[:], x_t[:], AF.Square,
                                             accum_out=ssq[:])
                    nm = pnrm.tile([128, 1], f32, tag="nm")
                    nc.scalar.activation(nm[:], ssq[:], AF.Sqrt)
                    rn = pnrm.tile([128, 1], f32, tag="rn")
                    nc.vector.reciprocal(rn[:], nm[:])
                    fa_t = pfa.tile([128, D], bf16, tag="fa", name=f"fa{rt}")
                    if rt % 2 == 0:
                        nc.scalar.activation(fa_t[:], x_t[:], AF.Copy,
                                             scale=rn[:])
                    else:
                        nc.vector.tensor_scalar(fa_t[:], x_t[:], rn[:], None,
                                                OP.mult)
                    fa2_t = pfa2.tile([128, D], bf16, tag="fa2",
                                      name=f"fa2{rt}")
                    nc.vector.tensor_tensor(fa2_t[:], fa_t[:], sg_t[:],
                                            OP.mult)
                    fas.append(fa_t)
                    fa2s.append(fa2_t)
                    if rt % 4 != 3:
                        continue
                    p = rt // 4
                    pma_ps = ppm.tile([128, 1], f32, tag="pmaps")
                    for s in range(2):
                        b = 2 * p + s
                        pv = ppb.tile([128, D], f32, tag="pbig")
                        for h in range(2):
                            for u in range(2):
                                rr = 2 * b + u
                                nc.tensor.matmul(
                                    pv[:, 512 * h:512 * (h + 1)],
                                    lhsT=wsa[:, 128 * rr:128 * (rr + 1)],
                                    rhs=fas[rr][:, 512 * h:512 * (h + 1)],
                                    start=(u == 0), stop=False)
                                nc.tensor.matmul(
                                    pv[:, 512 * h:512 * (h + 1)],
                                    lhsT=wsr[:, 128 * rr:128 * (rr + 1)],
                                    rhs=fa2s[rr][:, 512 * h:512 * (h + 1)],
                                    start=False, stop=(u == 1))
                        rowsq = pnrm.tile([128, 1], f32, tag="rowsq")
                        sq2 = psq.tile([128, D], bf16, tag="sqscr")
                        nc.scalar.activation(sq2[:], pv[:], AF.Square,
                                             accum_out=rowsq[:])
                        ea_t = pea.tile([128, D], fp8, tag="ea")
                        nc.vector.tensor_copy(ea_t[:], pv[:])
                        nc.sync.dma_start(
                            pay_all[EA0 + 128 * b:EA0 + 128 * (b + 1), :],
                            ea_t[:])
                        nc.tensor.matmul(pma_ps[:],
                                         lhsT=red2[:, 128 * s:128 * (s + 1)],
                                         rhs=rowsq[:],
                                         start=(s == 0), stop=(s == 1))
                    ma_c = ma4[:, p:p + 1]
                    nc.vector.tensor_tensor(ma_c, pma_ps[:],
                                            atab[:, 16 * p + 7:16 * p + 8],
                                            OP.mult)
                    ma2 = psmall.tile([128, 2], fp8, tag="ma2")
                    nc.vector.tensor_copy(ma2[:, 0:1], ma_c)
                    lo = pnrm.tile([128, 1], f32, tag="lo")
                    nc.vector.tensor_tensor(lo[:], ma_c, ma2[:, 0:1],
                                            OP.subtract)
                    nc.vector.tensor_copy(ma2[:, 1:2], lo[:])
                    nc.sync.dma_start(
                        pay_all[MA_ROW:MA_ROW + 1, 256 * p:256 * (p + 1)],
                        ma2[:])
                    gA_t = pga.tile([128, 2 * D], fp8, tag="ga", name=f"gA{p}")
                    nc.sync.dma_start(
                        gA_t[:],
                        pay_all[EA0 + 256 * p:EA0 + 256 * (p + 1), :].rearrange(
                            "(a x) c -> a (x c)", x=2))
                    gAs.append(gA_t)

            # ---- AllGather 1 (fc) ----
            nc.gpsimd.collective_compute(
                "AllGather", OP.bypass, replica_groups=rg,
                ins=[pay_all[0:EA0, :]], outs=[full_fc[:]])

            # ---- dist matmul + argmax + gathers + stats ----
            with ExitStack() as stC:
                psc = stC.enter_context(
                    tc.tile_pool(name="psc", bufs=4, space="PSUM"))

                rhsT = []
                for r in range(NCORES):
                    rt_ = prhs.tile([128, 4 * D], fp8, tag="rhs",
                                    name=f"rhsr{r}")
                    nc.sync.dma_start(
                        rt_[:].rearrange("p (j c) -> p j c", j=KT // 2),
                        full_fc[EA0 * r:EA0 * r + 512, :].rearrange(
                            "(b j) c -> b j c", j=KT // 2))
                    rhsT.append(rt_)

                gB, mi8s, ogbs, omas = [], [], [], []
                meta4 = pbat.tile([128, 8 * PT], fp8, tag="meta4")
                mab4 = pbat.tile([128, 2 * PT], fp8, tag="mab4")
                for mt in range(PT):
                    simp = [psc.tile([128, 1024], f32, tag="simp",
                                     name=f"simp{mt}_{c}")
                            for c in range(4)]
                    for ch in range(4):
                        for h in range(2):
                            nc.tensor.matmul(
                                simp[ch][:, 512 * h:512 * (h + 1)],
                                lhsT=maskI[:],
                                rhs=eqm[mt][:, 1024 * ch + 512 * h:
                                            1024 * ch + 512 * (h + 1)],
                                start=True, stop=False)
                    if USE_DR:
                        for j in range(KT // 2):
                            lhs3 = fcT[j][:].rearrange(
                                "p (k c) -> p k c", k=2)[:, :,
                                                         128 * mt:128 * (mt + 1)]
                            for ch in range(4):
                                for h in range(2):
                                    blk = 2 * ch + h
                                    rhs3 = rhsT[blk][:, 1024 * j:
                                                     1024 * (j + 1)].rearrange(
                                        "p (k c) -> p k c", k=2)
                                    nc.tensor.matmul(
                                        simp[ch][:, 512 * h:512 * (h + 1)],
                                        lhsT=lhs3, rhs=rhs3,
                                        start=False, stop=(j == KT // 2 - 1),
                                        perf_mode=DR)
                    else:
                        for j in range(KT // 2):
                            for half in range(2):
                                for ch in range(4):
                                    for h in range(2):
                                        blk = 2 * ch + h
                                        nc.tensor.matmul(
                                            simp[ch][:, 512 * h:512 * (h + 1)],
                                            lhsT=fcT[j][:, G * half + 128 * mt:
                                                        G * half + 128 * (mt + 1)],
                                            rhs=rhsT[blk][:, 1024 * j + 512 * half:
                                                          1024 * j + 512 * (half + 1)],
                                            start=False,
                                            stop=(j == KT // 2 - 1 and half == 1))
                    sim_t = psim.tile([128, NG], fp16, tag="sim",
                                      name=f"sim{mt}")
                    for ch in range(4):
                        nc.scalar.activation(
                            sim_t[:, 1024 * ch:1024 * (ch + 1)],
                            simp[ch][:], AF.Copy)
                    mx8 = psmall.tile([128, 8], fp16, tag="mx8")
                    mi8 = psmall.tile([128, 8], u32, tag="mi8")
                    nc.vector.max(mx8[:], sim_t[:])
                    nc.vector.max_index(mi8[:], mx8[:], sim_t[:])
                    idx = mi8[:, 0:1]
                    mi8s.append(mi8)

                    r9 = psmall.tile([128, 1], u32, tag="r9")
                    nc.vector.tensor_scalar(r9[:], idx, 9, None,
                                            OP.arith_shift_right)
                    o_gb = psmall.tile([128, 1], u32, tag="ogb",
                                       name=f"ogb{mt}")
                    nc.vector.tensor_scalar(o_gb[:], r9[:], GB_RM, GB_OFF,
                                            OP.mult, OP.add)
                    nc.vector.tensor_tensor(o_gb[:], o_gb[:], idx, OP.add)
                    o_ma = psmall.tile([128, 1], u32, tag="oma",
                                       name=f"oma{mt}")
                    nc.vector.tensor_scalar(o_ma[:], r9[:], MA_RM, MA_OFF,
                                            OP.mult, OP.add)
                    nc.vector.tensor_tensor(o_ma[:], o_ma[:], idx, OP.add)
                    ogbs.append(o_gb)
                    omas.append(o_ma)

                # ---- AllGather 2 (ea), emitted after dist ----
                nc.gpsimd.collective_compute(
                    "AllGather", OP.bypass, replica_groups=rg,
                    ins=[pay_all[EA0:PAY_ROWS, :]], outs=[full_ea[:]])
                for mt in range(PT):
                    gB_t = pgb.tile([128, 2 * D], fp8, tag="gb",
                                    name=f"gB{mt}")
                    nc.gpsimd.indirect_dma_start(
                        out=gB_t[:], out_offset=None,
                        in_=full_ea[:].rearrange("(a x) c -> a (x c)", x=2),
                        in_offset=bass.IndirectOffsetOnAxis(ap=ogbs[mt][:],
                                                            axis=0))
                    gB.append(gB_t)
                    nc.gpsimd.indirect_dma_start(
                        out=mab4[:, 2 * mt:2 * (mt + 1)], out_offset=None,
                        in_=full_ea[:].rearrange("a (c y) -> (a c) y", y=2),
                        in_offset=bass.IndirectOffsetOnAxis(ap=omas[mt][:],
                                                            axis=0))
                    nc.gpsimd.indirect_dma_start(
                        out=meta4[:, 8 * mt:8 * (mt + 1)], out_offset=None,
                        in_=gmeta[:],
                        in_offset=bass.IndirectOffsetOnAxis(
                            ap=mi8s[mt][:, 0:1], axis=0))

                # ---- per-tile dots, then batched stats ----
                dsa4 = pbat.tile([128, PT], f32, tag="dsa4")
                ds4 = pbat.tile([128, PT], f32, tag="ds4")
                cdot4 = pbat.tile([128, PT], f32, tag="cdot4")
                mb4 = pbat.tile([128, PT], f32, tag="mb4")
                for mt in range(PT):
                    scr = pscr.tile([128, D], bf16, tag="dotscr")
                    nc.vector.scalar_tensor_tensor(
                        scr[:], gAs[mt][:, 0:D], 1.0, gB[mt][:, 0:D],
                        OP.mult, OP.mult, accum_out=dsa4[:, mt:mt + 1])
                    scr2 = pscr.tile([128, D], bf16, tag="dotscr")
                    nc.vector.scalar_tensor_tensor(
                        scr2[:], gAs[mt][:, D:2 * D], 1.0, gB[mt][:, D:2 * D],
                        OP.mult, OP.mult, accum_out=ds4[:, mt:mt + 1])
                    scr6 = psmall.tile([128, 6], f32, tag="scr6")
                    nc.vector.scalar_tensor_tensor(
                        scr6[:], atab[:, 16 * mt:16 * mt + 6], 1.0,
                        meta4[:, 8 * mt:8 * mt + 6], OP.mult, OP.mult,
                        accum_out=cdot4[:, mt:mt + 1])
                    nc.vector.tensor_tensor(mb4[:, mt:mt + 1],
                                            mab4[:, 2 * mt:2 * mt + 1],
                                            mab4[:, 2 * mt + 1:2 * mt + 2],
                                            OP.add)

                vb4 = meta4[:].rearrange("p (m c) -> p m c", m=PT)[:, :, 6]
                B = psmall
                scross = B.tile([128, PT], f32, tag="scross")
                nc.vector.tensor_tensor(scross[:], dsa4[:], ds4[:],
                                        OP.subtract)
                c2 = B.tile([128, PT], f32, tag="c2")
                nc.vector.tensor_scalar(c2[:], cdot4[:], -2.0, 32.0,
                                        OP.mult, OP.add)
                c2m = B.tile([128, PT], f32, tag="c2m")
                nc.vector.tensor_scalar(c2m[:], c2[:], 1.0, None, OP.max)
                rec2 = B.tile([128, PT], f32, tag="rec2")
                nc.vector.reciprocal(rec2[:], c2m[:])
                valid2 = B.tile([128, PT], f32, tag="valid2")
                nc.vector.tensor_scalar(valid2[:], c2[:], 0.0, None, OP.is_gt)
                m2 = B.tile([128, PT], f32, tag="m2")
                nc.vector.scalar_tensor_tensor(m2[:], scross[:], -2.0,
                                               rec2[:], OP.mult, OP.mult)
                wa = B.tile([128, PT], f32, tag="wa")
                nc.vector.tensor_scalar(wa[:], vb4, -0.5, 1.0,
                                        OP.mult, OP.add)
                nc.vector.tensor_tensor(wa[:], wa[:], va4[:], OP.mult)
                wb = B.tile([128, PT], f32, tag="wb")
                nc.vector.tensor_scalar(wb[:], va4[:], -0.5, 1.0,
                                        OP.mult, OP.add)
                nc.vector.tensor_tensor(wb[:], wb[:], vb4, OP.mult)
                m1 = B.tile([128, PT], f32, tag="m1")
                nc.vector.tensor_tensor(m1[:], wa[:], ma4[:], OP.mult)
                m1b = B.tile([128, PT], f32, tag="m1b")
                nc.vector.tensor_tensor(m1b[:], wb[:], mb4[:], OP.mult)
                nc.vector.tensor_tensor(m1[:], m1[:], m1b[:], OP.add)
                diff = B.tile([128, PT], f32, tag="diff")
                nc.vector.scalar_tensor_tensor(diff[:], m1[:], MARGIN,
                                               m2[:], OP.add, OP.subtract)
                lossv = B.tile([128, PT], f32, tag="lossv")
                nc.scalar.activation(lossv[:], diff[:], AF.Relu)
                vor = B.tile([128, PT], f32, tag="vor")
                nc.vector.tensor_tensor(vor[:], va4[:], vb4, OP.mult)
                vsum = B.tile([128, PT], f32, tag="vsum")
                nc.vector.tensor_tensor(vsum[:], va4[:], vb4, OP.add)
                nc.vector.tensor_tensor(vor[:], vsum[:], vor[:], OP.subtract)
                nc.vector.tensor_tensor(lossv[:], lossv[:], vor[:], OP.mult)
                lossall = pbat.tile([128, PT], f32, tag="lossall")
                nc.vector.tensor_tensor(lossall[:], lossv[:], valid2[:],
                                        OP.mult)

                # cross-partition reduce via DRAM roundtrip
                for mt in range(PT):
                    nc.sync.dma_start(dscr[128 * mt:128 * (mt + 1), :],
                                      lossall[:, mt:mt + 1])
                lrow = psmall.tile([1, 4 * 128], f32, tag="lrow")
                nc.sync.dma_start(lrow[:], dscr[:].rearrange("a b -> b a"))
                lsb = psmall.tile([1, 1], f32, tag="lsb")
                nc.vector.tensor_reduce(lsb[:], lrow[:],
                                        mybir.AxisListType.X, OP.add)
                nc.sync.dma_start(loss_out[:], lsb[:])

    nc.compile()
    return nc


def _host_prep(input, target, camera_id):
    x = np.asarray(input, dtype=np.float32)
    tgt = np.asarray(target).reshape(NG, 4)
    cam = np.asarray(camera_id).reshape(NG, 4).astype(np.int64)
    labels = tgt[:, 0].astype(np.int64)

    rng = np.random.default_rng(1234)
    SGN = rng.choice([-1.0, 1.0], size=(6, D)).astype(np.float32)

    cnt = np.zeros((NG, 6), np.float32)
    for c in range(6):
        cnt[:, c] = (cam == c).sum(axis=1)
    c1 = 16.0 - (cnt * cnt).sum(axis=1)
    rec1 = (1.0 / np.maximum(c1, 1.0)).astype(np.float32)
    va = (c1 > 0).astype(np.float32)

    gmeta = np.zeros((NG, 8), np.float32)
    gmeta[:, 0:6] = cnt
    gmeta[:, 6] = va
    gmeta8 = gmeta.astype(ml_dtypes.float8_e4m3fn)

    e4 = np.zeros((4, 128, 128), np.float32)
    for j in range(4):
        for i in range(128):
            e4[j, i, 32 * j + i // 4] = 1.0
    e4b = np.ascontiguousarray(
        e4.transpose(1, 0, 2).reshape(128, 4 * 128)).astype(
        ml_dtypes.bfloat16)

    wsa = np.zeros((RT, 128, 128), np.float32)
    wsr = np.zeros((RT, 128, 128), np.float32)
    for rt in range(RT):
        for i in range(128):
            m = 64 * (rt % 2) + 2 * (i // 4)
            wsa[rt, i, m] = 1.0
            wsr[rt, i, m + 1] = 1.0
    wsa = np.ascontiguousarray(
        wsa.transpose(1, 0, 2).reshape(128, RT * 128))
    wsr = np.ascontiguousarray(
        wsr.transpose(1, 0, 2).reshape(128, RT * 128))

    red2 = np.zeros((2, 128, 128), np.float32)
    for s in range(2):
        for m in range(128):
            red2[s, m, 64 * s + m // 2] = 1.0 if (m % 2 == 1) else -1.0
    red2 = np.ascontiguousarray(
        red2.transpose(1, 0, 2).reshape(128, 256))

    ident = np.eye(128, dtype=ml_dtypes.bfloat16)
    maskI = np.eye(128, dtype=np.float32).astype(ml_dtypes.float8_e4m3fn)

    xb = x.astype(ml_dtypes.bfloat16)
    sgn_all = SGN[cam.reshape(-1)].astype(ml_dtypes.float8_e4m3fn)

    in_maps = []
    for k in range(NCORES):
        g0 = k * G
        eqm = np.zeros((PT, 128, NG), np.float32)
        atab = np.zeros((PT, 128, 16), np.float32)
        va4 = np.zeros((128, PT), np.float32)
        for p in range(PT):
            lg = g0 + 128 * p + np.arange(128)
            eqm[p] = MNEG * (labels[lg][:, None] == labels[None, :])
            atab[p, :, 0:6] = cnt[lg]
            atab[p, :, 6] = va[lg]
            atab[p, :, 7] = rec1[lg]
            va4[:, p] = va[lg]
        eqm = np.ascontiguousarray(
            eqm.transpose(1, 0, 2).reshape(128, PT * NG))
        atab = np.ascontiguousarray(
            atab.transpose(1, 0, 2).reshape(128, PT * 16))
        in_maps.append({
            "x_sh": xb[k * R:(k + 1) * R],
            "sgn": sgn_all[k * R:(k + 1) * R],
            "e4b": e4b,
            "wsa": wsa.astype(ml_dtypes.bfloat16),
            "wsr": wsr.astype(ml_dtypes.bfloat16),
            "red2": red2,
            "ident": ident,
            "maskI": maskI,
            "eqm": eqm.astype(ml_dtypes.float8_e4m3fn),
            "atab": atab,
            "va4": va4,
            "gmeta": gmeta8,
        })
    return in_maps


def kernel(input, target, camera_id):
    if "nc" not in _CACHE:
        _CACHE["nc"] = _build()
    nc = _CACHE["nc"]
    in_maps = _host_prep(input, target, camera_id)
    res = run_bass_kernel_spmd(nc, in_maps, core_ids=list(range(NCORES)))
    total = np.float64(0.0)
    for r in range(NCORES):
        total += np.float64(res.results[r]["loss_part"][0, 0])
    return np.float32(total)
